# revision 34
# baseline (speedup 1.0000x reference)
"""GQA causal self-attention on 8 Trainium2 NeuronCores.

Problem: B=2, T=2048, C=2048, H=16 query heads, HKV=4 kv heads, HD=128.
Sharding: core (b, g) for b in {0,1}, g in {0..3} owns batch b, kv head g,
and the 4 query heads h with h % 4 == g (reference's _expand_kv maps query
head h -> kv head h % HKV).  Each core computes its heads' attention output
and a partial output projection (its 512 rows of Wp); the host sums the 4
partials per batch and adds bp.  No cross-core communication on device.

Device math per core (all matmuls fp16 operands, fp32 PSUM accumulation):
  qT[d, t] = Wq_g.T @ x_b.T      (x is fed pre-transposed from host)
  kT[d, t] = Wk_g.T @ x_b.T
  v[t, d]  = x_b @ Wv_g          (lhsT = xT tiles)
  ST[j, i] = kT_j . qT_i         (j keys on partitions, i queries free)
  A = exp(ST / sqrt(HD)) masked causally: per 128-key tile the matmul
      free dim is clipped to i >= jtile start (exact block-causal), and
      a [128,128] lower-triangular mask handles the diagonal 128-block.
  den[*, i] = sum_j A[j, i]      (single ones[128,128] matmul per (it,h))
  yT[d, i] = (sum_j v[j, d] A[j, i]) / den[i]
  out[i, o] += yT.T @ Wp_g       (partial; host sums over g)
"""

import math
import os
from contextlib import ExitStack

import numpy as np

import concourse.bass as bass
import concourse.mybir as mybir
import concourse.tile as tile
from concourse import bacc, bass_utils

# The axon trace path needs antenv.axon_hooks; if the environment requests
# tracing but lacks the hook module, force tracing off instead of crashing.
if os.environ.get("BASS_TRACE"):
    try:
        import antenv.axon_hooks  # noqa: F401
    except ImportError:
        os.environ["BASS_NEVER_TRACE"] = "1"

# Problem shapes (hardcoded per contest rules).
B, T, C = 2, 2048, 2048
H, G = 16, 4
HKV = H // G          # 4 kv heads
HD = C // H           # 128 head dim
P = 128               # partitions
NH = H // HKV         # 4 local query heads per core
KT = C // P           # 16 contraction tiles for projections
TW = 512              # token tile width (matmul free dim)
NT = T // TW          # 4 token tiles
JTN = T // P          # 16 key tiles of 128
SCALE = 1.0 / math.sqrt(HD)

FP = mybir.dt.float16
F32 = mybir.dt.float32

_CACHE = {}

# Set by kernel() after each run: bass_utils.BassKernelResults.
LAST_RESULT = None


def _build_bass():
    nc = bacc.Bacc("TRN2")

    xt = nc.dram_tensor("xt", [C, T], FP, kind="ExternalInput")
    wq = nc.dram_tensor("wq", [C, NH * HD], FP, kind="ExternalInput")
    wk = nc.dram_tensor("wk", [C, HD], FP, kind="ExternalInput")
    wv = nc.dram_tensor("wv", [C, HD], FP, kind="ExternalInput")
    wp = nc.dram_tensor("wp", [NH * HD, C], FP, kind="ExternalInput")
    bq = nc.dram_tensor("bq", [NH * HD], F32, kind="ExternalInput")
    bk = nc.dram_tensor("bk", [HD], F32, kind="ExternalInput")
    bv = nc.dram_tensor("bv", [HD], F32, kind="ExternalInput")
    mask = nc.dram_tensor("mask", [P, P], FP, kind="ExternalInput")
    out = nc.dram_tensor("out", [T, C], F32, kind="ExternalOutput")

    xt_r = xt.ap().rearrange("(ko p) t -> p ko t", p=P)       # [128,16,2048]
    wq_r = wq.ap().rearrange("(ko p) m -> p ko m", p=P)       # [128,16,512]
    wk_r = wk.ap().rearrange("(ko p) m -> p ko m", p=P)       # [128,16,128]
    wv_r = wv.ap().rearrange("(ko p) m -> p ko m", p=P)
    wp_r = wp.ap().rearrange("(h p) o -> p h o", p=P)         # [128,4,2048]
    bq_r = bq.ap().rearrange("(h p) -> p h", p=P)             # [128,4]
    out_r = out.ap().rearrange("(io p) o -> p io o", p=P)     # [128,16,2048]

    with tile.TileContext(nc) as tc, ExitStack() as ctx:
        consts = ctx.enter_context(tc.tile_pool(name="consts", bufs=1))
        xpool = ctx.enter_context(tc.tile_pool(name="xpool", bufs=2))
        espool = ctx.enter_context(tc.tile_pool(name="espool", bufs=8))
        mpool = ctx.enter_context(tc.tile_pool(name="mpool", bufs=2))
        opool = ctx.enter_context(tc.tile_pool(name="opool", bufs=3))
        # PSUM (8 banks), all single-bank [P,512] tiles:
        #   ps_s 3x (S per key-tile pipeline; also 3 of the q-proj accums)
        #   ps_y 2x (AV accumulator double-buffered across heads; also k-proj)
        #   ps_d 1x (softmax denominator; also the 4th q-proj accum)
        #   ps_o 2x (out-proj column slices; also v-proj, warmup)
        ps_s = ctx.enter_context(tc.tile_pool(name="ps_s", bufs=3, space="PSUM"))
        ps_y = ctx.enter_context(tc.tile_pool(name="ps_y", bufs=2, space="PSUM"))
        ps_d = ctx.enter_context(tc.tile_pool(name="ps_d", bufs=1, space="PSUM"))
        ps_o = ctx.enter_context(tc.tile_pool(name="ps_o", bufs=2, space="PSUM"))

        # First-needed weights + first x tile, in fine chunks so the first
        # real q matmul's inputs arrive as early as possible.
        KC = 4
        wq_sb = consts.tile([P, KT, NH * HD], FP)
        wk_sb = consts.tile([P, KT, HD], FP)
        wv_sb = consts.tile([P, KT, HD], FP)
        xtile0 = xpool.tile([P, KT, TW], FP, tag="xt", name="xtile0")
        # First-tile loads alternate across the sync and gpsimd DMA-issue
        # queues (two parallel serialized streams), with each k-chunk's xt
        # and wq on opposite queues so both arrive in step.
        KC0 = 8
        for c8 in range(KC0):
            ks = slice(c8 * (KT // KC0), (c8 + 1) * (KT // KC0))
            ex, ew = (nc.sync, nc.gpsimd) if c8 % 2 == 0 else (nc.gpsimd, nc.sync)
            ex.dma_start(out=xtile0[:, ks], in_=xt_r[:, ks, 0:TW])
            ew.dma_start(out=wq_sb[:, ks], in_=wq_r[:, ks])
        bq_sb = consts.tile([P, NH], F32)
        nc.gpsimd.dma_start(out=bq_sb, in_=bq_r)
        ones_sb = consts.tile([P, P], FP)
        nc.vector.memset(ones_sb, 1.0)
        dummy_sb = consts.tile([P, P], FP)
        nc.vector.memset(dummy_sb, 0.0)

        # PE warm-up: HAM un-throttles (1.2 -> 2.4 GHz) after ~3.4us of
        # sustained matmul activity.  Short throwaway matmuls keep the PE
        # busy from kernel start until the first input chunk lands (~10.3us)
        # so the real matmuls run at full clock from their first cycle.
        ps_warm = ps_o.tile([P, TW], F32, tag="pso", name="ps_warm")
        for w in range(48):
            nc.tensor.matmul(
                ps_warm[:, :P],
                lhsT=ones_sb,
                rhs=dummy_sb,
                start=True,
                stop=True,
            )

        # Persistent activations.
        qT = consts.tile([P, NH, T], FP)       # [d, h, i]
        kT = consts.tile([P, T], FP)           # [d, j]
        v_sb = consts.tile([P, JTN, HD], FP)   # [j_in, j_tile, d]
        yT = consts.tile([P, NH, T], FP)       # [d, h, i]

        # ---- Projections ----
        for n in range(NT):
            if n == 0:
                xtile = xtile0
            else:
                xtile = xpool.tile([P, KT, TW], FP, tag="xt", name=f"xtile{n}")
                for c2 in range(2):
                    ks = slice(c2 * (KT // 2), (c2 + 1) * (KT // 2))
                    nc.sync.dma_start(
                        out=xtile[:, ks], in_=xt_r[:, ks, n * TW:(n + 1) * TW]
                    )
            # q: 4 head accumulators, one PSUM bank each (3 from ps_s, the
            # 4th from ps_y whose other buffer holds the k-proj accumulator).
            psq = [
                ps_s.tile([P, TW], F32, tag="ps1", name=f"psq_{n}_{h}")
                for h in range(NH - 1)
            ] + [ps_y.tile([P, TW], F32, tag="psy", name=f"psq_{n}_3")]
            for k in range(KT):
                for h in range(NH):
                    nc.tensor.matmul(
                        psq[h],
                        lhsT=wq_sb[:, k, h * HD:(h + 1) * HD],
                        rhs=xtile[:, k, :],
                        start=(k == 0),
                        stop=(k == KT - 1),
                    )
            if n == 0:
                # k/v weights land while the n=0 q matmuls run.
                for c4 in range(KC):
                    ks = slice(c4 * (KT // KC), (c4 + 1) * (KT // KC))
                    nc.gpsimd.dma_start(out=wk_sb[:, ks], in_=wk_r[:, ks])
                    nc.gpsimd.dma_start(out=wv_sb[:, ks], in_=wv_r[:, ks])
                bk_sb = consts.tile([P, 1], F32)
                nc.gpsimd.dma_start(
                    out=bk_sb, in_=bk.ap().rearrange("(h p) -> p h", p=P)
                )
                # bv broadcast across partitions (partition step 0 source).
                bv_bc = consts.tile([P, HD], F32)
                bv_ap = bass.AP(tensor=bv.ap().tensor, offset=0, ap=[[0, P], [1, HD]])
                nc.gpsimd.dma_start(out=bv_bc, in_=bv_ap)
            for h in range(NH):
                nc.vector.tensor_scalar(
                    out=qT[:, h, n * TW:(n + 1) * TW],
                    in0=psq[h],
                    scalar1=bq_sb[:, h:h + 1],
                    scalar2=None,
                    op0=mybir.AluOpType.add,
                )
            psk = ps_y.tile([P, TW], F32, tag="psy", name=f"psk_{n}")
            for k in range(KT):
                nc.tensor.matmul(
                    psk,
                    lhsT=wk_sb[:, k, :],
                    rhs=xtile[:, k, :],
                    start=(k == 0),
                    stop=(k == KT - 1),
                )
            nc.vector.tensor_scalar(
                out=kT[:, n * TW:(n + 1) * TW],
                in0=psk,
                scalar1=bk_sb,
                scalar2=None,
                op0=mybir.AluOpType.add,
            )
            # v-projection: 4 key-tiles of 128 tokens per n.  The n=3 groups
            # (needed only at it=3) are deferred into the it=0 attention loop
            # as exp-independent PE filler.
            def v_group(n, js, xtile=xtile):
                psv = ps_o.tile([P, TW], F32, tag="pso", name=f"psv_{n}_{js}")
                for k in range(KT):
                    nc.tensor.matmul(
                        psv[:, :HD],
                        lhsT=xtile[:, k, js * P:(js + 1) * P],
                        rhs=wv_sb[:, k, :],
                        start=(k == 0),
                        stop=(k == KT - 1),
                    )
                jt = n * (TW // P) + js
                nc.vector.tensor_tensor(
                    out=v_sb[:, jt, :],
                    in0=psv[:, :HD],
                    in1=bv_bc,
                    op=mybir.AluOpType.add,
                )

            if n < NT - 1:
                for js in range(TW // P):
                    v_group(n, js)
            else:
                deferred_v = [
                    (lambda js=js, xt3=xtile: v_group(NT - 1, js, xtile=xt3))
                    for js in range(TW // P)
                ]

        # Weights for the later phases: load after projection work is queued.
        wp_sb = consts.tile([P, NH, C], FP)
        nc.gpsimd.dma_start(out=wp_sb, in_=wp_r)
        mask_sb = consts.tile([P, P], FP)
        nc.gpsimd.dma_start(out=mask_sb, in_=mask.ap())

        # ---- Attention with interleaved output projection ----
        # Out-proj for i-chunk ic is computed in 4 column slices of 512, each
        # a single PSUM bank; the PSUM->SBUF copies alternate between the
        # scalar and vector engines so neither becomes the bottleneck, and
        # each 512-slice is stored as soon as its copy lands.  Slices are
        # emitted as individual ops from a fill queue, spread through the
        # attention tile loop so the PE always has exp-independent work while
        # the scalar engine (whose exp rate exactly matches S+AV) catches up.
        ncopy = [0]

        def chunk_slice(ic, os_, osb):
            pso = ps_o.tile([P, TW], F32, tag="pso", name=f"pso_{ic}_{os_}")
            for h in range(NH):
                nc.tensor.matmul(
                    pso,
                    lhsT=yT[:, h, ic * P:(ic + 1) * P],
                    rhs=wp_sb[:, h, os_ * TW:(os_ + 1) * TW],
                    start=(h == 0),
                    stop=(h == NH - 1),
                )
            osl = slice(os_ * TW, (os_ + 1) * TW)
            if ncopy[0] % 2 == 0:
                nc.scalar.copy(out=osb[:, osl], in_=pso)
            else:
                nc.vector.tensor_copy(out=osb[:, osl], in_=pso)
            ncopy[0] += 1
            # Stores go out on the (otherwise idle) gpsimd queue so the
            # sync engine's serialized DMA-issue stream never delays them.
            nc.gpsimd.dma_start(out=out_r[:, ic, osl], in_=osb[:, osl])

        def chunk_ops(ic):
            osb = opool.tile([P, C], F32, tag="osb", name=f"osb_{ic}")
            return [
                (lambda os_=os_: chunk_slice(ic, os_, osb))
                for os_ in range(C // TW)
            ]

        def out_proj_chunk(ic):
            for op in chunk_ops(ic):
                op()

        # The whole normalization chain of head (it,h) — den matmul, recip,
        # divide — is emitted only after the NEXT head's attention stream, so
        # the PE's den matmul never waits on the freshest acc adds (which are
        # gated by the last exp), and no later matmul inherits the chain via
        # the counting-semaphore dependency model.  `pending` holds
        # (acc, psy, h, i0); emit_den/emit_div split it for the tail filler.
        pending = [None]

        def emit_den():
            acc, psy, h, i0 = pending[0]
            psd = ps_d.tile([P, TW], F32, tag="psd", name=f"psd_{h}_{i0}")
            nc.tensor.matmul(psd, lhsT=ones_sb, rhs=acc, start=True, stop=True)
            return psd

        def emit_div(psd):
            acc, psy, h, i0 = pending[0]
            pending[0] = None
            rb = mpool.tile([P, TW], F32, tag="rb")
            nc.vector.reciprocal_approx_fast(out=rb, in_=psd)
            nc.vector.tensor_mul(yT[:, h, i0:i0 + TW], psy, rb)

        def flush_pending():
            if pending[0] is not None:
                emit_div(emit_den())

        for it in range(NT):
            i0 = it * TW
            nfull = 4 * it          # full 128-key tiles below the diagonal
            for h in range(NH):
                # Fill work for this head, spread through the tile loop.  The
                # chunk schedule lags one head so no fill op ever reads a yT
                # block whose divide has not been emitted yet:
                #   (it,h>=1) -> chunk (it-1)*4 + h-1
                #   (it,0)    -> chunk (it-2)*4 + 3
                #   it=0      -> the deferred n=3 v-projection groups
                # Chunk 11 is reserved as the PE filler after den(3,3).
                fill = []
                if it == 0:
                    if h >= 1:
                        fill = [deferred_v[h - 1]]
                elif it == 1 and h == 0:
                    fill = [deferred_v[3]]
                elif h >= 1:
                    fill = chunk_ops((it - 1) * (TW // P) + h - 1)
                else:
                    fill = chunk_ops((it - 2) * (TW // P) + 3)
                psy = ps_y.tile([P, TW], F32, tag="psy", name=f"psy_{it}_{h}")
                acc = mpool.tile([P, TW], FP, tag="acc", name=f"acc_{it}_{h}")
                ntile = nfull + 4
                stride = max(1, ntile // max(1, len(fill)))
                fpos = {s * stride for s in range(len(fill))}
                fi = 0
                for ti in range(ntile):
                    if ti < nfull:
                        jt = ti
                        off = 0          # i-offset into this 512-block
                        w = TW
                    else:
                        kd = ti - nfull  # diagonal 128-tile index
                        jt = nfull + kd
                        off = kd * P
                        w = TW - off
                    pss = ps_s.tile(
                        [P, TW], F32, tag="ps1", name=f"pss_{it}_{h}_{ti}"
                    )
                    nc.tensor.matmul(
                        pss[:, :w],
                        lhsT=kT[:, jt * P:(jt + 1) * P],
                        rhs=qT[:, h, i0 + off:i0 + TW],
                        start=True,
                        stop=True,
                    )
                    es = espool.tile([P, TW], FP, tag="es")
                    nc.scalar.activation(
                        out=es[:, :w],
                        in_=pss[:, :w],
                        func=mybir.ActivationFunctionType.Exp,
                        scale=SCALE,
                    )
                    if ti >= nfull:
                        # Mask the leading 128 columns (the diagonal block).
                        nc.vector.tensor_mul(es[:, :P], es[:, :P], mask_sb)
                    nc.tensor.matmul(
                        psy[:, off:],
                        lhsT=v_sb[:, jt, :],
                        rhs=es[:, :w],
                        start=(ti == 0),
                        stop=(ti == ntile - 1),
                    )
                    # Fill op before this tile's acc add, so it only waits on
                    # DVE work that is already done.  Emitted early in each
                    # stride window, where the head's exp pipeline is priming
                    # and the PE would otherwise run ahead of the scalar
                    # engine.
                    if fi < len(fill) and ti in fpos:
                        fill[fi]()
                        fi += 1
                    if ti == 0:
                        nc.vector.tensor_copy(out=acc, in_=es)
                    else:
                        nc.vector.tensor_tensor(
                            out=acc[:, off:],
                            in0=acc[:, off:],
                            in1=es[:, :w],
                            op=mybir.AluOpType.add,
                        )
                while fi < len(fill):
                    fill[fi]()
                    fi += 1
                # Normalization chain of the PREVIOUS head, hidden under this
                # head's attention stream.
                flush_pending()
                pending[0] = (acc, psy, h, i0)
        # Tail: den(3,3) first, then the reserved chunk 11 as PE filler while
        # the DVE runs recip+divide(3,3); the tail chunks then flow without a
        # cold PE window.
        psd_last = emit_den()
        out_proj_chunk(11)
        emit_div(psd_last)
        for h in range(NH):
            out_proj_chunk((NT - 1) * (TW // P) + h)

    nc.compile()
    return nc


def kernel(x, Wkv, bkv, Wq, bq, Wp, bp):
    global LAST_RESULT
    x = np.asarray(x, np.float32)
    Wkv = np.asarray(Wkv, np.float32)
    bkv = np.asarray(bkv, np.float32)
    Wq = np.asarray(Wq, np.float32)
    bq = np.asarray(bq, np.float32)
    Wp = np.asarray(Wp, np.float32)
    bp = np.asarray(bp, np.float32)

    if "nc" not in _CACHE:
        _CACHE["nc"] = _build_bass()
    nc = _CACHE["nc"]

    # [128,128] lower-triangular causal mask for the diagonal blocks.
    mask = (np.arange(P)[:, None] <= np.arange(P)[None, :]).astype(np.float16)
    CG = C // G  # 512 columns per kv head in the k/v halves of Wkv

    in_maps = []
    for b in range(B):
        xt = x[b].T.astype(np.float16)
        for g in range(HKV):
            heads = [g + HKV * u for u in range(NH)]  # h % HKV == g
            wq_g = np.concatenate(
                [Wq[:, h * HD:(h + 1) * HD] for h in heads], axis=1
            ).astype(np.float16)
            bq_g = np.concatenate([bq[h * HD:(h + 1) * HD] for h in heads])
            wp_g = np.ascontiguousarray(
                np.concatenate([Wp[h * HD:(h + 1) * HD, :] for h in heads], axis=0)
            ).astype(np.float16)
            wk_g = np.ascontiguousarray(Wkv[:, g * HD:(g + 1) * HD]).astype(np.float16)
            wv_g = np.ascontiguousarray(
                Wkv[:, CG + g * HD:CG + (g + 1) * HD]
            ).astype(np.float16)
            bk_g = np.ascontiguousarray(bkv[g * HD:(g + 1) * HD])
            bv_g = np.ascontiguousarray(bkv[CG + g * HD:CG + (g + 1) * HD])
            in_maps.append(
                {
                    "xt": xt,
                    "wq": wq_g,
                    "wk": wk_g,
                    "wv": wv_g,
                    "wp": wp_g,
                    "bq": np.ascontiguousarray(bq_g, np.float32),
                    "bk": np.ascontiguousarray(bk_g, np.float32),
                    "bv": np.ascontiguousarray(bv_g, np.float32),
                    "mask": mask,
                }
            )

    res = bass_utils.run_bass_kernel_spmd(nc, in_maps, core_ids=list(range(B * HKV)))
    LAST_RESULT = res

    out = np.zeros((B, T, C), np.float32)
    for b in range(B):
        acc = np.zeros((T, C), np.float32)
        for g in range(HKV):
            acc += res.results[b * HKV + g]["out"]
        out[b] = acc + bp[None, :]
    return out
